# revision 9
# baseline (speedup 1.0000x reference)
"""TRN2 Bass kernel: 3x3 valid cross-correlation + bias on [8192, 8192] fp32.

Sharding: image columns across 8 NeuronCores (1026-col input shards with the
2-col halo included in each host-side slice; weight/bias replicated).

Per-core compute: the 3x3 conv is computed on the tensor engine as banded
matmuls. For a stripe of 128 input rows (SBUF partitions) the column (dy)
taps become a 3-banded [128, 126] stationary matrix B with B[k, m] = w[k-m, dx],
and the row (dx) taps become free-dim shifts of the moving operand:

    out[m, n] = sum_dx (B_dx.T @ X[:, n+dx])[m]   (3 matmuls, PSUM-accumulated)

fp32 matmuls run at 4 cycles/row on TRN2 but float32r runs at 1 cycle/row,
so each fp32 matmul is decomposed into 3 fp32r matmuls via an exact split
X = Xr + Xe (fp32r rounding + fp32r residual) and W = Wr + We:

    W@X ~= Wr@Xr + Wr@Xe + We@Xr      (dropped We@Xe term ~2^-28 relative)

which is fp32-accurate and 1.33x faster than native fp32 matmuls. The vector
engine additionally computes the dx=2 term of some chunks directly in fp32
(scalar_tensor_tensor FMAs into PSUM) to offload the tensor engine.
"""
import os
import numpy as np
from contextlib import ExitStack

import concourse.bass as bass
import concourse.tile as tile
from concourse import mybir, bacc
from concourse.bass_utils import run_bass_kernel_spmd

H = W = 8192
KH = KW = 3
OH, OW = H - KH + 1, W - KW + 1          # 8190 x 8190
NCORES = 8
SHARD_OW = 1024                           # output cols per core
SHARD_IW = SHARD_OW + KW - 1              # 1026 input cols per core
STRIPE = 126                              # output rows per stripe (K=128 band)
NSTRIPES = OH // STRIPE                   # 65, exact
CHUNK = 512                               # matmul moving free dim (PSUM bank)
NCHUNKS = SHARD_OW // CHUNK               # 2

F32 = mybir.dt.float32
F32R = mybir.dt.float32r
I32 = mybir.dt.int32
OP = mybir.AluOpType


def build_nc(sb_bufs=3, ps_bufs=6):
    nc = bacc.Bacc("TRN2", target_bir_lowering=False, debug=False,
                   num_devices=NCORES)
    x_sh = nc.dram_tensor("x_sh", [H, SHARD_IW], F32, kind="ExternalInput").ap()
    w_d = nc.dram_tensor("w", [KH, KW], F32, kind="ExternalInput").ap()
    b_d = nc.dram_tensor("b", [1], F32, kind="ExternalInput").ap()
    out_sh = nc.dram_tensor("out_sh", [OH, SHARD_OW], F32,
                            kind="ExternalOutput").ap()

    with tile.TileContext(nc) as tc, ExitStack() as ctx:
        consts = ctx.enter_context(tc.tile_pool(name="consts", bufs=1))
        xin = ctx.enter_context(tc.tile_pool(name="xin", bufs=sb_bufs))
        xr_p = ctx.enter_context(tc.tile_pool(name="xr", bufs=sb_bufs))
        xe_p = ctx.enter_context(tc.tile_pool(name="xe", bufs=sb_bufs))
        outp = ctx.enter_context(tc.tile_pool(name="outp", bufs=sb_bufs))
        psum = ctx.enter_context(tc.tile_pool(name="psum", bufs=ps_bufs,
                                              space="PSUM"))

        # ---- one-time setup: broadcast weights, build band matrices ----
        # wb[:, j] = w[j//3, j%3] for all partitions; wb[:, 9] = bias
        wb = consts.tile([128, 10], F32)
        nc.sync.dma_start(wb[:, 0:9], w_d.rearrange("a b -> (a b)")
                          .unsqueeze(0).partition_broadcast(128))
        nc.sync.dma_start(wb[:, 9:10], b_d.unsqueeze(0).partition_broadcast(128))
        wr = consts.tile([128, 10], F32R)    # rounded weights
        we = consts.tile([128, 10], F32R)    # weight residuals
        nc.scalar.copy(wr[:], wb[:])
        nc.vector.tensor_tensor(we[:], wb[:], wr[:].bitcast(F32), OP.subtract)

        # diag[p, m] = p - m ; mask_dy = (diag == dy)
        diag = consts.tile([128, STRIPE], I32)
        nc.gpsimd.iota(diag[:], pattern=[[-1, STRIPE]], base=0,
                       channel_multiplier=1)
        masks = []
        for dy in range(KH):
            m = consts.tile([128, STRIPE], F32, tag=f"mask{dy}")
            nc.vector.tensor_scalar(m[:], diag[:], dy, None, OP.is_equal)
            masks.append(m)
        # band matrices: br_dx[k, m] = wr[k-m, dx], be_dx[k, m] = we[k-m, dx]
        br, be = [], []
        for dx in range(KW):
            tr = consts.tile([128, STRIPE], F32R, tag=f"br{dx}")
            te = consts.tile([128, STRIPE], F32R, tag=f"be{dx}")
            for src, dst in ((wr, tr), (we, te)):
                nc.vector.tensor_scalar(dst[:], masks[0][:],
                                        src[:, dx:dx + 1].bitcast(F32), None,
                                        OP.mult)
                for dy in range(1, KH):
                    j = 3 * dy + dx
                    nc.vector.scalar_tensor_tensor(
                        dst[:], masks[dy][:], src[:, j:j + 1].bitcast(F32),
                        dst[:].bitcast(F32), OP.mult, OP.add)
            br.append(tr)
            be.append(te)
        bias_col = wb[0:STRIPE, 9:10]

        # ---- main loop over row stripes ----
        chunk_idx = 0
        for s in range(NSTRIPES):
            r0 = s * STRIPE
            xt = xin.tile([128, SHARD_IW], F32)
            nc.sync.dma_start(xt[:], x_sh[r0:r0 + 128, :])
            xr = xr_p.tile([128, SHARD_IW], F32R)
            xe = xe_p.tile([128, SHARD_IW], F32R)
            nc.scalar.copy(xr[:], xt[:])
            nc.vector.tensor_tensor(xe[:], xt[:], xr[:].bitcast(F32),
                                    OP.subtract)
            ot = outp.tile([STRIPE, SHARD_OW], F32)
            for c in range(NCHUNKS):
                n0 = c * CHUNK
                pt = psum.tile([STRIPE, CHUNK], F32)
                passes = []
                for dx in range(KW):
                    sl = slice(n0 + dx, n0 + dx + CHUNK)
                    passes += [(br[dx], xr[:, sl]), (br[dx], xe[:, sl]),
                               (be[dx], xr[:, sl])]
                for i, (lhsT, rhs) in enumerate(passes):
                    nc.tensor.matmul(pt[:], lhsT[:, :STRIPE], rhs,
                                     start=(i == 0), stop=(i == len(passes) - 1))
                nc.scalar.activation(ot[:, n0:n0 + CHUNK], pt[:],
                                     mybir.ActivationFunctionType.Identity,
                                     bias=bias_col)
                chunk_idx += 1
            nc.scalar.dma_start(out_sh[r0:r0 + STRIPE, :], ot[:])
    nc.compile()
    return nc


_nc_cache = {}


def _get_nc(**kw):
    key = tuple(sorted(kw.items()))
    if key not in _nc_cache:
        _nc_cache[key] = build_nc(**kw)
    return _nc_cache[key]


def shard_inputs(x, weight, bias):
    x = np.ascontiguousarray(np.asarray(x, dtype=np.float32))
    weight = np.ascontiguousarray(np.asarray(weight, dtype=np.float32))
    bias = np.ascontiguousarray(np.asarray(bias, dtype=np.float32))
    col0 = [min(c * SHARD_OW, W - SHARD_IW) for c in range(NCORES)]
    in_maps = [{"x_sh": np.ascontiguousarray(x[:, c0:c0 + SHARD_IW]),
                "w": weight, "b": bias} for c0 in col0]
    return in_maps, col0


def unshard_outputs(results, col0):
    out = np.empty((OH, OW), dtype=np.float32)
    for c in range(NCORES):
        sh = results[c]["out_sh"]
        lo = c * SHARD_OW
        hi = min(lo + SHARD_OW, OW)
        off = lo - col0[c]
        out[:, lo:hi] = sh[:, off:off + (hi - lo)]
    return out


def kernel(x, weight, bias, **build_kw):
    nc = _get_nc(**build_kw)
    in_maps, col0 = shard_inputs(x, weight, bias)
    res = run_bass_kernel_spmd(nc, in_maps, list(range(NCORES)))
    return unshard_outputs(res.results, col0)
